# revision 22
# baseline (speedup 1.0000x reference)
"""GRU-D decoder kernel for Trainium2 (8 NeuronCores, data-parallel over batch).

Math (mask == ones everywhere, which the reference hardcodes):
  x_hat = C (constant), d = dt broadcast, gamma_x unused.
  gamma[t,b,j] = exp(-relu(dt[t,b] * colsum(Wgh)[j] + bgh[j]))   (precomputed host-side)
  per step: hdec = gamma_t * h
            z = sigmoid(hdec @ Wz_h + Az0);  r = sigmoid(hdec @ Wr_h + Ar0)
            htl = tanh((r*hdec) @ Wh_h + Ah0)
            h = hdec + z*(htl - hdec)
  out[t] = h_t @ Wlin + blin
  where A?0 = C @ W?_x + colsum(W?_m) + b?  (time-constant, precomputed host-side).

Device layout: transposed (H on partitions as 4 tiles of 128, batch=64 on the
free dim), packed as SBUF tiles (128, 4*64) with column index = kt*64 + b.

Final design (707us baseline -> ~442us):
  * The recurrent state is bf16 end-to-end (validated vs the 2e-2 gate; the
    numpy quantization sim predicts hardware exactly): gamma ships bf16, the
    decayed state hdb is the single state tensor, and the blend
    h = z*htl + (hdec - z*hdec) runs with all-bf16 DVE operands (2x element
    rate), with the (hdec - z*hdec) term precomputed on GpSimd off the tanh
    critical path.  h is written directly into the bf16 projection archive
    slot, so there is no fp32 h, no hdf, and no archive copy.
  * r, z, and candidate PSUM tiles are allocated as full 2KB banks --- PSUM
    tiles smaller than a bank get packed two-per-bank, which serializes a
    new accumulation group against the previous step's activation reads of
    the co-resident tile and stalls the PE at every step boundary.
  * The output projection is batched over 2-step chunks (512-col archive ring)
    with wlin 128x128 stationary blocks; pieces (2 matmuls, 128-wide moving)
    are drip-fed 4 per step boundary as PE filler.  Pieces and the r/z bias
    matmuls carry explicit no-sync ordering edges anchored to the previous
    step's last candidate matmul --- without the anchor the Tile scheduler
    hoists all dependency-free work early and leaves the PE starved at the
    boundary while the tanh/blend/decay tail completes.  PSUM drains + blin
    add run on DVE, off the PE path.
  * Gate matmul waves are ordered to minimize the recurrent chain: r runs
    (kc0,kc1) x all jo then (kc2,kc3) by jo pairs; candidate runs kc-major
    with the half-0 stop first so tanh half 0 starts before the last
    candidate matmuls finish; the half-0 tanh/blend/decay chain is emitted
    under tc.high_priority() so the next step's decayed state never queues
    behind half-1 work on DVE.
  * Output is written [O, T, BL] per core (partition-major, contiguous DMA
    lines); the host transposes back.
"""

import numpy as np
import ml_dtypes

T, B, H, O = 100, 512, 512, 512
NCORES = 8
BL = B // NCORES  # 64
KC = 4   # contraction chunks of 128
JT = 4   # output j-tiles of 128
FR = JT * BL  # 256
HB = FR // 2  # 128
GCH = 20  # gamma chunk (steps per DMA)
CH = 2    # projection chunk (steps)

_BUILD_CACHE = {}


def _build_program():
    if "nc" in _BUILD_CACHE:
        return _BUILD_CACHE["nc"]

    import concourse.tile as tile
    import concourse.mybir as mybir
    from concourse import bacc
    from contextlib import ExitStack
    import bass_rust

    def _anchor_set(name):
        return bass_rust.InstructionNameOrderedSet([name])

    f32 = mybir.dt.float32
    bf16 = mybir.dt.bfloat16
    AF = mybir.ActivationFunctionType

    nc = bacc.Bacc("TRN2", target_bir_lowering=False, debug=False,
                   num_devices=NCORES)

    gam_d = nc.dram_tensor("gam", [128, T, FR], bf16, kind="ExternalInput")
    wzr_d = nc.dram_tensor("wzr", [128, KC * 2 * JT * 128], bf16, kind="ExternalInput")
    wht_d = nc.dram_tensor("wht", [128, KC * JT * 128], bf16, kind="ExternalInput")
    wlin_d = nc.dram_tensor("wlin", [128, KC * JT * 128], bf16, kind="ExternalInput")
    a0zr_d = nc.dram_tensor("a0zr", [128, 2 * FR], bf16, kind="ExternalInput")
    a0h_d = nc.dram_tensor("a0h", [128, FR], bf16, kind="ExternalInput")
    ident_d = nc.dram_tensor("ident", [128, 128], bf16, kind="ExternalInput")
    blinc_d = nc.dram_tensor("blinc", [128, JT], f32, kind="ExternalInput")
    out_d = nc.dram_tensor("out", [O, T, BL], f32, kind="ExternalOutput")

    with tile.TileContext(nc) as tc, ExitStack() as ctx:
        constp = ctx.enter_context(tc.tile_pool(name="const", bufs=1))
        gpool = ctx.enter_context(tc.tile_pool(name="gam", bufs=2))
        hbigp = ctx.enter_context(tc.tile_pool(name="hbig", bufs=3))
        hdp = ctx.enter_context(tc.tile_pool(name="hd", bufs=2))
        actp = ctx.enter_context(tc.tile_pool(name="act", bufs=2))
        osbp = ctx.enter_context(tc.tile_pool(name="osb", bufs=2))
        prp = ctx.enter_context(tc.tile_pool(name="pr", bufs=1, space="PSUM"))
        pzp = ctx.enter_context(tc.tile_pool(name="pz", bufs=1, space="PSUM"))
        candp = ctx.enter_context(tc.tile_pool(name="cand", bufs=2, space="PSUM"))
        pjp = ctx.enter_context(tc.tile_pool(name="pj", bufs=4, space="PSUM"))

        wzr = constp.tile([128, KC * 2 * JT * 128], bf16)
        nc.sync.dma_start(wzr[:], wzr_d[:])
        wht = constp.tile([128, KC * JT * 128], bf16)
        nc.sync.dma_start(wht[:], wht_d[:])
        wlin = constp.tile([128, KC * JT * 128], bf16)
        nc.sync.dma_start(wlin[:], wlin_d[:])
        a0zr = constp.tile([128, 2 * FR], bf16)
        nc.sync.dma_start(a0zr[:], a0zr_d[:])
        a0h = constp.tile([128, FR], bf16)
        nc.sync.dma_start(a0h[:], a0h_d[:])
        ident = constp.tile([128, 128], bf16)
        nc.sync.dma_start(ident[:], ident_d[:])
        blinc = constp.tile([128, JT], f32)
        nc.sync.dma_start(blinc[:], blinc_d[:])

        def wzr_blk(g, jo, kc):
            i = ((kc * 2 + g) * JT + jo) * 128
            return wzr[:, i:i + 128]

        def wht_blk(jo, kc):
            i = (kc * JT + jo) * 128
            return wht[:, i:i + 128]

        def wlin_blk(jo, kc):
            i = (kc * JT + jo) * 128
            return wlin[:, i:i + 128]

        # gamma chunks, preloaded half a chunk ahead
        chunks = {}

        def ensure_chunk(c):
            if c in chunks or c * GCH >= T:
                return
            t0 = c * GCH
            t1 = min(t0 + GCH, T)
            gt = gpool.tile([128, GCH * FR], bf16, tag="gchunk")
            nc.sync.dma_start(gt[:, 0:(t1 - t0) * FR], gam_d[:, t0:t1, :])
            chunks[c] = gt

        def gamma_sl(tt, c0, c1):
            c2, o2 = divmod(tt, GCH)
            return chunks[c2][:, o2 * FR + c0: o2 * FR + c1]

        ensure_chunk(0)

        # step-0 decayed state is zero
        hdb = hdp.tile([128, FR], bf16, tag="hdb")
        nc.vector.memset(hdb[:], 0.0)

        # ---- projection machinery ------------------------------------------
        hbig_tiles = {}        # chunk index -> archive tile [128, CH, FR]
        pj_tiles = {}          # (chunk, jo) -> psum tile
        pieces = []            # pending (chunk, jo, half) matmul pieces
        drains = []            # pending (chunk, jo) PSUM drains

        def n_steps(c):
            return min(CH, T - c * CH)

        def emit_piece(c, jo, half, anchor=None):
            n = n_steps(c)
            if (c, jo) not in pj_tiles:
                pj_tiles[(c, jo)] = pjp.tile([128, 512], f32, tag="pj",
                                             name=f"pj_{c}_{jo}")
            pj = pj_tiles[(c, jo)]
            hb = hbig_tiles[c]
            for kc in (2 * half, 2 * half + 1):
                mm = nc.tensor.matmul(
                    pj[:, 0:n * BL],
                    wlin_blk(jo, kc),
                    hb[:, 0:n, kc * BL:(kc + 1) * BL],
                    start=(kc == 0), stop=(kc == 3),
                )
                if anchor is not None:
                    mm.ins.add_nosync_dependencies_from(_anchor_set(anchor))
            if half == 1:
                drains.append((c, jo))

        def emit_drain(c, jo):
            n = n_steps(c)
            pj = pj_tiles.pop((c, jo))
            osb = osbp.tile([128, CH * BL], f32, tag="osb")
            nc.vector.tensor_single_scalar(
                osb[:, 0:n * BL], pj[:, 0:n * BL],
                blinc[:, jo:jo + 1], mybir.AluOpType.add)
            nc.sync.dma_start(
                out_d[jo * 128:(jo + 1) * 128, c * CH:c * CH + n, :],
                osb[:, 0:n * BL])

        anchor = None
        for t in range(T):
            c, o = divmod(t, GCH)
            if o == GCH // 2:
                ensure_chunk(c + 1)

            pc, s = divmod(t, CH)
            if s == 0:
                hbig_tiles[pc] = hbigp.tile([128, CH, FR], bf16, tag="hbig",
                                            name=f"hbig_{pc}")
            hb = hbig_tiles[pc]

            # ---- boundary filler: projection pieces + bias inits, anchored
            # after the previous step's candidate matmuls so the scheduler
            # cannot hoist them away from the boundary.
            for _ in range(3):
                if pieces and t >= (pieces[0][0] + 1) * CH + 1:
                    emit_piece(*pieces.pop(0), anchor=anchor)
                else:
                    break
            pr = prp.tile([128, 512], f32, tag="pr")
            mm = nc.tensor.matmul(pr[:, 0:FR], ident[:], a0zr[:, 0:FR], start=True, stop=False)
            if anchor is not None:
                mm.ins.add_nosync_dependencies_from(_anchor_set(anchor))
            pz = pzp.tile([128, 512], f32, tag="pz")
            mm = nc.tensor.matmul(pz[:, 0:FR], ident[:], a0zr[:, FR:2 * FR], start=True, stop=False)
            if anchor is not None:
                mm.ins.add_nosync_dependencies_from(_anchor_set(anchor))

            # ---- r gate: (kc0,kc1) x all jo, then (kc2,kc3) by jo pairs so
            # sigmoid halves can start as early as possible
            def gate_mm(g, jo, kc):
                tgt = pr if g == 0 else pz
                return nc.tensor.matmul(
                    tgt[:, jo * BL:(jo + 1) * BL],
                    wzr_blk(g, jo, kc),
                    hdb[:, kc * BL:(kc + 1) * BL],
                    start=False, stop=(kc == KC - 1),
                )

            for kc in (0, 1):
                for jo in range(JT):
                    gate_mm(0, jo, kc)
            for jo in (0, 1):
                for kc in (2, 3):
                    gate_mm(0, jo, kc)
            for jo in (2, 3):
                for kc in (2, 3):
                    gate_mm(0, jo, kc)
            rb = actp.tile([128, FR], bf16, tag="rb")
            nc.scalar.activation(rb[:], pr[:, 0:FR], AF.Sigmoid)

            # ---- z gate (fills the PE while sigmoid(r)/rh run elsewhere)
            for kc in range(KC):
                for jo in range(JT):
                    zmm = gate_mm(1, jo, kc)
            z_anchor = zmm.ins.name
            zb = actp.tile([128, FR], bf16, tag="zb")

            if pieces and t >= (pieces[0][0] + 1) * CH + 1:
                emit_piece(*pieces.pop(0), anchor=z_anchor)
            candt = candp.tile([128, 512], f32, tag="candt")
            mm = nc.tensor.matmul(candt[:, 0:FR], ident[:], a0h[:], start=True, stop=False)
            mm.ins.add_nosync_dependencies_from(_anchor_set(z_anchor))

            rh = hdp.tile([128, FR], bf16, tag="rh")
            nc.vector.tensor_mul(rh[:, 0:HB], rb[:, 0:HB], hdb[:, 0:HB])
            nc.vector.tensor_mul(rh[:, HB:FR], rb[:, HB:FR], hdb[:, HB:FR])

            # ---- candidate: kc-major waves; half-0 (jo 0,1) stops first
            for kc in (0, 1):
                for jo in range(JT):
                    nc.tensor.matmul(
                        candt[:, jo * BL:(jo + 1) * BL],
                        wht_blk(jo, kc), rh[:, kc * BL:(kc + 1) * BL],
                        start=False, stop=False)
            for jo in (0, 1):
                for kc in (2, 3):
                    nc.tensor.matmul(
                        candt[:, jo * BL:(jo + 1) * BL],
                        wht_blk(jo, kc), rh[:, kc * BL:(kc + 1) * BL],
                        start=False, stop=(kc == 3))
            for jo in (2, 3):
                for kc in (2, 3):
                    mm = nc.tensor.matmul(
                        candt[:, jo * BL:(jo + 1) * BL],
                        wht_blk(jo, kc), rh[:, kc * BL:(kc + 1) * BL],
                        start=False, stop=(kc == 3))
            anchor = mm.ins.name

            nc.scalar.activation(zb[:], pz[:, 0:FR], AF.Sigmoid)

            # ---- tail: h = z*htl + (hdec - z*hdec).  The second term is
            # precomputed off the tanh critical path; all-bf16 DVE ops.
            pq = actp.tile([128, FR], bf16, tag="pq")
            nc.gpsimd.tensor_mul(pq[:], zb[:], hdb[:])
            gg = actp.tile([128, FR], bf16, tag="gg")
            nc.gpsimd.tensor_sub(gg[:], hdb[:], pq[:])
            htl = actp.tile([128, FR], bf16, tag="htl")
            qq = actp.tile([128, FR], bf16, tag="qq")
            hdb_n = None
            if t + 1 < T:
                hdb_n = hdp.tile([128, FR], bf16, tag="hdb")
            for hf in (0, 1):
                sl = slice(hf * HB, (hf + 1) * HB)
                from contextlib import nullcontext
                with (tc.high_priority() if hf == 0 else nullcontext()):
                    act_last = nc.scalar.activation(htl[:, sl], candt[:, sl], AF.Tanh)
                    nc.vector.tensor_mul(qq[:, sl], zb[:, sl], htl[:, sl])
                    dve_last = nc.vector.tensor_add(hb[:, s, sl], qq[:, sl], gg[:, sl])
                    if t + 1 < T:
                        dve_last = nc.vector.tensor_mul(
                            hdb_n[:, sl], gamma_sl(t + 1, hf * HB, (hf + 1) * HB),
                            hb[:, s, sl])
            if t + 1 < T:
                hdb = hdb_n

            if s == CH - 1 or t == T - 1:
                for jo in range(JT):
                    pieces.append((pc, jo, 0))
                    pieces.append((pc, jo, 1))

            # ---- PSUM drains for completed projection pieces (off-chain)
            while drains:
                emit_drain(*drains.pop(0))

        # drain remaining projection pieces
        while pieces:
            emit_piece(*pieces.pop(0))
        while drains:
            emit_drain(*drains.pop(0))

    nc.compile()
    _BUILD_CACHE["nc"] = nc
    return nc


def _host_prep(C, t, Wz, bz, Wr, br, Wh, bh, Wgh, bgh, Wlin, blin):
    """Build per-core input maps (all the precomputed, packed device tensors)."""
    bf = ml_dtypes.bfloat16

    s = Wgh.sum(axis=0)  # (H,)
    t3 = t[:, :, 0]  # (T,B)
    dt = np.concatenate([np.zeros((1, B), np.float32), t3[1:] - t3[:-1]], axis=0)
    # gamma (T,B,H)
    gam = np.exp(-np.maximum(dt[:, :, None] * s[None, None, :] + bgh[None, None, :], 0.0)).astype(np.float32)

    def gate_const(W, b):
        # C @ W_x + colsum(W_m) + b  -> (B,H)
        return C @ W[0:H] + (W[2 * H:3 * H].sum(axis=0) + b)[None, :]

    Az0 = gate_const(Wz, bz).astype(np.float32)
    Ar0 = gate_const(Wr, br).astype(np.float32)
    Ah0 = gate_const(Wh, bh).astype(np.float32)

    Wg = np.stack([Wr[H:2 * H], Wz[H:2 * H]])  # (2,H,H): g=0 -> r, g=1 -> z
    # wzr packed: [k, (kc,g,jo,m)]
    wzr = Wg.reshape(2, KC, 128, JT, 128).transpose(2, 1, 0, 3, 4).reshape(128, KC * 2 * JT * 128)
    wht = Wh[H:2 * H].reshape(KC, 128, JT, 128).transpose(1, 0, 2, 3).reshape(128, KC * JT * 128)
    # wlin packed: [k, (kc,jo,m)] with block (kc,jo) = Wlin[kc*128:(kc+1)*128, jo*128:(jo+1)*128]
    wlin = Wlin.reshape(KC, 128, JT, 128).transpose(1, 0, 2, 3).reshape(128, KC * JT * 128)
    wzr = np.ascontiguousarray(wzr, dtype=bf)
    wht = np.ascontiguousarray(wht, dtype=bf)
    wlin = np.ascontiguousarray(wlin, dtype=bf)
    ident = np.eye(128, dtype=bf)
    blinc = np.ascontiguousarray(blin.reshape(JT, 128).T, dtype=np.float32)  # [128, JT]

    in_maps = []
    for i in range(NCORES):
        sl = slice(i * BL, (i + 1) * BL)
        gf = gam[:, sl, :]  # (T,BL,H)
        # gam packed: [p, t, kt*BL+b]
        gp = np.ascontiguousarray(
            gf.reshape(T, BL, KC, 128).transpose(3, 0, 2, 1).reshape(128, T, KC * BL),
            dtype=bf)

        def packA(A):
            return A[sl].reshape(BL, JT, 128).transpose(2, 1, 0).reshape(128, JT * BL)

        a0zr = np.concatenate([packA(Ar0), packA(Az0)], axis=1)

        in_maps.append({
            "gam": gp,
            "wzr": wzr,
            "wht": wht,
            "wlin": wlin,
            "a0zr": np.ascontiguousarray(a0zr, dtype=bf),
            "a0h": np.ascontiguousarray(packA(Ah0), dtype=bf),
            "ident": ident,
            "blinc": blinc,
        })
    return in_maps


def kernel(C, t, mask, Wz, bz, Wr, br, Wh, bh, Wgh, bgh, wgx, bgx, Wlin, blin,
           _trace=False, _trace_kwargs=None):
    C = np.asarray(C, np.float32)
    t = np.asarray(t, np.float32)
    nc = _build_program()
    in_maps = _host_prep(C, t,
                         np.asarray(Wz, np.float32), np.asarray(bz, np.float32),
                         np.asarray(Wr, np.float32), np.asarray(br, np.float32),
                         np.asarray(Wh, np.float32), np.asarray(bh, np.float32),
                         np.asarray(Wgh, np.float32), np.asarray(bgh, np.float32),
                         np.asarray(Wlin, np.float32), np.asarray(blin, np.float32))

    from concourse.bass_utils import run_bass_kernel_spmd
    res = run_bass_kernel_spmd(nc, in_maps, list(range(NCORES)),
                               trace=_trace, **(_trace_kwargs or {}))
    # per-core out is [O, T, BL] -> (T, BL, O)
    outs = [res.results[i]["out"].transpose(1, 2, 0) for i in range(NCORES)]
    full = np.concatenate(outs, axis=1).astype(np.float32)  # (T,B,O)
    kernel._last_results = res
    return full


# revision 24
# speedup vs baseline: 1.1861x; 1.1861x over previous
"""GRU-D decoder kernel for Trainium2 (8 NeuronCores, data-parallel over batch).

Math (mask == ones everywhere, which the reference hardcodes):
  x_hat = C (constant), d = dt broadcast, gamma_x unused.
  gamma[t,b,j] = exp(-relu(dt[t,b] * colsum(Wgh)[j] + bgh[j]))   (precomputed host-side)
  per step: hdec = gamma_t * h
            z = sigmoid(hdec @ Wz_h + Az0);  r = sigmoid(hdec @ Wr_h + Ar0)
            htl = tanh((r*hdec) @ Wh_h + Ah0)
            h = hdec + z*(htl - hdec)
  out[t] = h_t @ Wlin + blin
  where A?0 = C @ W?_x + colsum(W?_m) + b?  (time-constant, precomputed host-side).

Device layout: transposed (H on partitions as 4 tiles of 128, batch=64 on the
free dim), packed as SBUF tiles (128, 4*64) with column index = kt*64 + b.

Final design (707us baseline -> ~442us):
  * The recurrent state is bf16 end-to-end (validated vs the 2e-2 gate; the
    numpy quantization sim predicts hardware exactly): gamma ships bf16, the
    decayed state hdb is the single state tensor, and the blend
    h = z*htl + (hdec - z*hdec) runs with all-bf16 DVE operands (2x element
    rate), with the (hdec - z*hdec) term precomputed on GpSimd off the tanh
    critical path.  h is written directly into the bf16 projection archive
    slot, so there is no fp32 h, no hdf, and no archive copy.
  * r, z, and candidate PSUM tiles are allocated as full 2KB banks --- PSUM
    tiles smaller than a bank get packed two-per-bank, which serializes a
    new accumulation group against the previous step's activation reads of
    the co-resident tile and stalls the PE at every step boundary.
  * The output projection is batched over 2-step chunks (512-col archive ring)
    with wlin 128x128 stationary blocks; pieces (2 matmuls, 128-wide moving)
    are drip-fed 4 per step boundary as PE filler.  Pieces and the r/z bias
    matmuls carry explicit no-sync ordering edges anchored to the previous
    step's last candidate matmul --- without the anchor the Tile scheduler
    hoists all dependency-free work early and leaves the PE starved at the
    boundary while the tanh/blend/decay tail completes.  PSUM drains + blin
    add run on DVE, off the PE path.
  * Gate matmul waves are ordered to minimize the recurrent chain: r runs
    (kc0,kc1) x all jo then (kc2,kc3) by jo pairs; candidate runs kc-major
    with the half-0 stop first so tanh half 0 starts before the last
    candidate matmuls finish; the half-0 tanh/blend/decay chain is emitted
    under tc.high_priority() so the next step's decayed state never queues
    behind half-1 work on DVE.
  * Output is written [O, T, BL] per core (partition-major, contiguous DMA
    lines); the host transposes back.
"""

import numpy as np
import ml_dtypes

T, B, H, O = 100, 512, 512, 512
NCORES = 8
BL = B // NCORES  # 64
KC = 4   # contraction chunks of 128
JT = 4   # output j-tiles of 128
FR = JT * BL  # 256
HB = FR // 2  # 128
GCH = 50  # gamma chunk (steps per DMA)
CH = 2    # projection chunk (steps)

_BUILD_CACHE = {}


def _build_program():
    if "nc" in _BUILD_CACHE:
        return _BUILD_CACHE["nc"]

    import concourse.tile as tile
    import concourse.mybir as mybir
    from concourse import bacc
    from contextlib import ExitStack
    import bass_rust

    def _anchor_set(name):
        return bass_rust.InstructionNameOrderedSet([name])

    f32 = mybir.dt.float32
    bf16 = mybir.dt.bfloat16
    AF = mybir.ActivationFunctionType

    nc = bacc.Bacc("TRN2", target_bir_lowering=False, debug=False,
                   num_devices=NCORES)

    gam_d = nc.dram_tensor("gam", [128, T, FR], bf16, kind="ExternalInput")
    wzr_d = nc.dram_tensor("wzr", [128, KC * 2 * JT * 128], bf16, kind="ExternalInput")
    wht_d = nc.dram_tensor("wht", [128, KC * JT * 128], bf16, kind="ExternalInput")
    wlin_d = nc.dram_tensor("wlin", [128, KC * JT * 128], bf16, kind="ExternalInput")
    a0zr_d = nc.dram_tensor("a0zr", [128, 2 * FR], bf16, kind="ExternalInput")
    a0h_d = nc.dram_tensor("a0h", [128, FR], bf16, kind="ExternalInput")
    ident_d = nc.dram_tensor("ident", [128, 128], bf16, kind="ExternalInput")
    blinc_d = nc.dram_tensor("blinc", [128, JT], f32, kind="ExternalInput")
    out_d = nc.dram_tensor("out", [O, T, BL], f32, kind="ExternalOutput")

    with tile.TileContext(nc) as tc, ExitStack() as ctx:
        constp = ctx.enter_context(tc.tile_pool(name="const", bufs=1))
        gpool = ctx.enter_context(tc.tile_pool(name="gam", bufs=2))
        hbigp = ctx.enter_context(tc.tile_pool(name="hbig", bufs=3))
        hdp = ctx.enter_context(tc.tile_pool(name="hd", bufs=2))
        actp = ctx.enter_context(tc.tile_pool(name="act", bufs=2))
        osbp = ctx.enter_context(tc.tile_pool(name="osb", bufs=2))
        prp = ctx.enter_context(tc.tile_pool(name="pr", bufs=1, space="PSUM"))
        pzp = ctx.enter_context(tc.tile_pool(name="pz", bufs=1, space="PSUM"))
        candp = ctx.enter_context(tc.tile_pool(name="cand", bufs=2, space="PSUM"))
        pjp = ctx.enter_context(tc.tile_pool(name="pj", bufs=4, space="PSUM"))

        wzr = constp.tile([128, KC * 2 * JT * 128], bf16)
        nc.sync.dma_start(wzr[:], wzr_d[:])
        wht = constp.tile([128, KC * JT * 128], bf16)
        nc.sync.dma_start(wht[:], wht_d[:])
        wlin = constp.tile([128, KC * JT * 128], bf16)
        nc.sync.dma_start(wlin[:], wlin_d[:])
        a0zr = constp.tile([128, 2 * FR], bf16)
        nc.sync.dma_start(a0zr[:], a0zr_d[:])
        a0h = constp.tile([128, FR], bf16)
        nc.sync.dma_start(a0h[:], a0h_d[:])
        ident = constp.tile([128, 128], bf16)
        nc.sync.dma_start(ident[:], ident_d[:])
        blinc = constp.tile([128, JT], f32)
        nc.sync.dma_start(blinc[:], blinc_d[:])

        def wzr_blk(g, jo, kc):
            i = ((kc * 2 + g) * JT + jo) * 128
            return wzr[:, i:i + 128]

        def wht_blk(jo, kc):
            i = (kc * JT + jo) * 128
            return wht[:, i:i + 128]

        def wlin_blk(jo, kc):
            i = (kc * JT + jo) * 128
            return wlin[:, i:i + 128]

        # gamma chunks, preloaded half a chunk ahead
        chunks = {}

        def ensure_chunk(c):
            if c in chunks or c * GCH >= T:
                return
            t0 = c * GCH
            t1 = min(t0 + GCH, T)
            gt = gpool.tile([128, GCH * FR], bf16, tag="gchunk")
            nc.sync.dma_start(gt[:, 0:(t1 - t0) * FR], gam_d[:, t0:t1, :])
            chunks[c] = gt

        def gamma_sl(tt, c0, c1):
            c2, o2 = divmod(tt, GCH)
            return chunks[c2][:, o2 * FR + c0: o2 * FR + c1]

        ensure_chunk(0)

        # step-0 decayed state is zero
        hdb = hdp.tile([128, FR], bf16, tag="hdb")
        nc.vector.memset(hdb[:], 0.0)

        # ---- projection machinery ------------------------------------------
        hbig_tiles = {}        # chunk index -> archive tile [128, CH, FR]
        pj_tiles = {}          # (chunk, jo) -> psum tile
        pieces = []            # pending (chunk, jo, half) matmul pieces
        drains = []            # pending (chunk, jo) PSUM drains

        def n_steps(c):
            return min(CH, T - c * CH)

        def emit_piece(c, jo, half, anchor=None):
            n = n_steps(c)
            if (c, jo) not in pj_tiles:
                pj_tiles[(c, jo)] = pjp.tile([128, 512], f32, tag="pj",
                                             name=f"pj_{c}_{jo}")
            pj = pj_tiles[(c, jo)]
            hb = hbig_tiles[c]
            for kc in (2 * half, 2 * half + 1):
                mm = nc.tensor.matmul(
                    pj[:, 0:n * BL],
                    wlin_blk(jo, kc),
                    hb[:, 0:n, kc * BL:(kc + 1) * BL],
                    start=(kc == 0), stop=(kc == 3),
                )
                if anchor is not None:
                    mm.ins.add_nosync_dependencies_from(_anchor_set(anchor))
            if half == 1:
                drains.append((c, jo))

        def emit_drain(c, jo):
            n = n_steps(c)
            pj = pj_tiles.pop((c, jo))
            osb = osbp.tile([128, CH * BL], f32, tag="osb")
            nc.vector.tensor_single_scalar(
                osb[:, 0:n * BL], pj[:, 0:n * BL],
                blinc[:, jo:jo + 1], mybir.AluOpType.add)
            nc.sync.dma_start(
                out_d[jo * 128:(jo + 1) * 128, c * CH:c * CH + n, :],
                osb[:, 0:n * BL])

        anchor = None
        for t in range(T):
            c, o = divmod(t, GCH)
            if o == GCH // 2:
                ensure_chunk(c + 1)

            pc, s = divmod(t, CH)
            if s == 0:
                hbig_tiles[pc] = hbigp.tile([128, CH, FR], bf16, tag="hbig",
                                            name=f"hbig_{pc}")
            hb = hbig_tiles[pc]

            # ---- boundary filler: projection pieces + bias inits, anchored
            # after the previous step's candidate matmuls so the scheduler
            # cannot hoist them away from the boundary.
            for _ in range(4):
                if pieces and t >= (pieces[0][0] + 1) * CH + 1:
                    emit_piece(*pieces.pop(0), anchor=anchor)
                else:
                    break
            pr = prp.tile([128, 512], f32, tag="pr")
            mm = nc.tensor.matmul(pr[:, 0:FR], ident[:], a0zr[:, 0:FR], start=True, stop=False)
            if anchor is not None:
                mm.ins.add_nosync_dependencies_from(_anchor_set(anchor))
            pz = pzp.tile([128, 512], f32, tag="pz")
            mm = nc.tensor.matmul(pz[:, 0:FR], ident[:], a0zr[:, FR:2 * FR], start=True, stop=False)
            if anchor is not None:
                mm.ins.add_nosync_dependencies_from(_anchor_set(anchor))

            # ---- r gate: (kc0,kc1) x all jo, then (kc2,kc3) by jo pairs so
            # sigmoid halves can start as early as possible
            def gate_mm(g, jo, kc):
                tgt = pr if g == 0 else pz
                nc.tensor.matmul(
                    tgt[:, jo * BL:(jo + 1) * BL],
                    wzr_blk(g, jo, kc),
                    hdb[:, kc * BL:(kc + 1) * BL],
                    start=False, stop=(kc == KC - 1),
                )

            for kc in (0, 1):
                for jo in range(JT):
                    gate_mm(0, jo, kc)
            for jo in (0, 1):
                for kc in (2, 3):
                    gate_mm(0, jo, kc)
            for jo in (2, 3):
                for kc in (2, 3):
                    gate_mm(0, jo, kc)
            rb = actp.tile([128, FR], bf16, tag="rb")
            nc.scalar.activation(rb[:], pr[:, 0:FR], AF.Sigmoid)

            # ---- z gate (fills the PE while sigmoid(r)/rh run elsewhere)
            for kc in range(KC):
                for jo in range(JT):
                    gate_mm(1, jo, kc)
            zb = actp.tile([128, FR], bf16, tag="zb")

            candt = candp.tile([128, 512], f32, tag="candt")
            nc.tensor.matmul(candt[:, 0:FR], ident[:], a0h[:], start=True, stop=False)

            rh = hdp.tile([128, FR], bf16, tag="rh")
            nc.vector.tensor_mul(rh[:, 0:HB], rb[:, 0:HB], hdb[:, 0:HB])
            nc.vector.tensor_mul(rh[:, HB:FR], rb[:, HB:FR], hdb[:, HB:FR])

            # ---- candidate: kc-major waves; half-0 (jo 0,1) stops first
            for kc in (0, 1):
                for jo in range(JT):
                    nc.tensor.matmul(
                        candt[:, jo * BL:(jo + 1) * BL],
                        wht_blk(jo, kc), rh[:, kc * BL:(kc + 1) * BL],
                        start=False, stop=False)
            for jo in (0, 1):
                for kc in (2, 3):
                    nc.tensor.matmul(
                        candt[:, jo * BL:(jo + 1) * BL],
                        wht_blk(jo, kc), rh[:, kc * BL:(kc + 1) * BL],
                        start=False, stop=(kc == 3))
            for jo in (2, 3):
                for kc in (2, 3):
                    mm = nc.tensor.matmul(
                        candt[:, jo * BL:(jo + 1) * BL],
                        wht_blk(jo, kc), rh[:, kc * BL:(kc + 1) * BL],
                        start=False, stop=(kc == 3))
            anchor = mm.ins.name

            nc.scalar.activation(zb[:], pz[:, 0:FR], AF.Sigmoid)

            # ---- tail: h = z*htl + (hdec - z*hdec).  The second term is
            # precomputed off the tanh critical path; all-bf16 DVE ops.
            pq = actp.tile([128, FR], bf16, tag="pq")
            nc.gpsimd.tensor_mul(pq[:], zb[:], hdb[:])
            gg = actp.tile([128, FR], bf16, tag="gg")
            nc.gpsimd.tensor_sub(gg[:], hdb[:], pq[:])
            htl = actp.tile([128, FR], bf16, tag="htl")
            qq = actp.tile([128, FR], bf16, tag="qq")
            hdb_n = None
            if t + 1 < T:
                hdb_n = hdp.tile([128, FR], bf16, tag="hdb")
            for hf in (0, 1):
                sl = slice(hf * HB, (hf + 1) * HB)
                from contextlib import nullcontext
                with (tc.high_priority() if hf == 0 else nullcontext()):
                    act_last = nc.scalar.activation(htl[:, sl], candt[:, sl], AF.Tanh)
                    nc.vector.tensor_mul(qq[:, sl], zb[:, sl], htl[:, sl])
                    dve_last = nc.vector.tensor_add(hb[:, s, sl], qq[:, sl], gg[:, sl])
                    if t + 1 < T:
                        dve_last = nc.vector.tensor_mul(
                            hdb_n[:, sl], gamma_sl(t + 1, hf * HB, (hf + 1) * HB),
                            hb[:, s, sl])
            if t + 1 < T:
                hdb = hdb_n

            if s == CH - 1 or t == T - 1:
                for jo in range(JT):
                    pieces.append((pc, jo, 0))
                    pieces.append((pc, jo, 1))

            # ---- PSUM drains for completed projection pieces (off-chain)
            while drains:
                emit_drain(*drains.pop(0))

        # drain remaining projection pieces
        while pieces:
            emit_piece(*pieces.pop(0))
        while drains:
            emit_drain(*drains.pop(0))

    nc.compile()
    _BUILD_CACHE["nc"] = nc
    return nc


def _host_prep(C, t, Wz, bz, Wr, br, Wh, bh, Wgh, bgh, Wlin, blin):
    """Build per-core input maps (all the precomputed, packed device tensors)."""
    bf = ml_dtypes.bfloat16

    s = Wgh.sum(axis=0)  # (H,)
    t3 = t[:, :, 0]  # (T,B)
    dt = np.concatenate([np.zeros((1, B), np.float32), t3[1:] - t3[:-1]], axis=0)
    # gamma (T,B,H)
    gam = np.exp(-np.maximum(dt[:, :, None] * s[None, None, :] + bgh[None, None, :], 0.0)).astype(np.float32)

    def gate_const(W, b):
        # C @ W_x + colsum(W_m) + b  -> (B,H)
        return C @ W[0:H] + (W[2 * H:3 * H].sum(axis=0) + b)[None, :]

    Az0 = gate_const(Wz, bz).astype(np.float32)
    Ar0 = gate_const(Wr, br).astype(np.float32)
    Ah0 = gate_const(Wh, bh).astype(np.float32)

    Wg = np.stack([Wr[H:2 * H], Wz[H:2 * H]])  # (2,H,H): g=0 -> r, g=1 -> z
    # wzr packed: [k, (kc,g,jo,m)]
    wzr = Wg.reshape(2, KC, 128, JT, 128).transpose(2, 1, 0, 3, 4).reshape(128, KC * 2 * JT * 128)
    wht = Wh[H:2 * H].reshape(KC, 128, JT, 128).transpose(1, 0, 2, 3).reshape(128, KC * JT * 128)
    # wlin packed: [k, (kc,jo,m)] with block (kc,jo) = Wlin[kc*128:(kc+1)*128, jo*128:(jo+1)*128]
    wlin = Wlin.reshape(KC, 128, JT, 128).transpose(1, 0, 2, 3).reshape(128, KC * JT * 128)
    wzr = np.ascontiguousarray(wzr, dtype=bf)
    wht = np.ascontiguousarray(wht, dtype=bf)
    wlin = np.ascontiguousarray(wlin, dtype=bf)
    ident = np.eye(128, dtype=bf)
    blinc = np.ascontiguousarray(blin.reshape(JT, 128).T, dtype=np.float32)  # [128, JT]

    in_maps = []
    for i in range(NCORES):
        sl = slice(i * BL, (i + 1) * BL)
        gf = gam[:, sl, :]  # (T,BL,H)
        # gam packed: [p, t, kt*BL+b]
        gp = np.ascontiguousarray(
            gf.reshape(T, BL, KC, 128).transpose(3, 0, 2, 1).reshape(128, T, KC * BL),
            dtype=bf)

        def packA(A):
            return A[sl].reshape(BL, JT, 128).transpose(2, 1, 0).reshape(128, JT * BL)

        a0zr = np.concatenate([packA(Ar0), packA(Az0)], axis=1)

        in_maps.append({
            "gam": gp,
            "wzr": wzr,
            "wht": wht,
            "wlin": wlin,
            "a0zr": np.ascontiguousarray(a0zr, dtype=bf),
            "a0h": np.ascontiguousarray(packA(Ah0), dtype=bf),
            "ident": ident,
            "blinc": blinc,
        })
    return in_maps


def kernel(C, t, mask, Wz, bz, Wr, br, Wh, bh, Wgh, bgh, wgx, bgx, Wlin, blin,
           _trace=False, _trace_kwargs=None):
    C = np.asarray(C, np.float32)
    t = np.asarray(t, np.float32)
    nc = _build_program()
    in_maps = _host_prep(C, t,
                         np.asarray(Wz, np.float32), np.asarray(bz, np.float32),
                         np.asarray(Wr, np.float32), np.asarray(br, np.float32),
                         np.asarray(Wh, np.float32), np.asarray(bh, np.float32),
                         np.asarray(Wgh, np.float32), np.asarray(bgh, np.float32),
                         np.asarray(Wlin, np.float32), np.asarray(blin, np.float32))

    from concourse.bass_utils import run_bass_kernel_spmd
    res = run_bass_kernel_spmd(nc, in_maps, list(range(NCORES)),
                               trace=_trace, **(_trace_kwargs or {}))
    # per-core out is [O, T, BL] -> (T, BL, O)
    outs = [res.results[i]["out"].transpose(1, 2, 0) for i in range(NCORES)]
    full = np.concatenate(outs, axis=1).astype(np.float32)  # (T,B,O)
    kernel._last_results = res
    return full
